# revision 10
# baseline (speedup 1.0000x reference)
"""ModalityUntiedAttention on 8 TRN2 NeuronCores (Bass/Tile) — v2.

Sharding: data-parallel over batch (cores 0-3 -> batch 0, cores 4-7 -> batch 1),
tensor-parallel over heads within each 4-core group (4 q heads + 2 kv heads per
core).

Expert (modality) routing: tokens are sorted by modality WITHIN each 512-token
attention group (host-side permutation), then routed at 128-token-tile
granularity with ZERO on-device select ops: a tile that is pure in both
batches runs one expert's weight chain; a tile that straddles the modality
boundary in either batch runs BOTH experts' chains accumulating into the same
PSUM, with the host zeroing each chain's x columns for tokens that belong to
the other expert (exact routing, compiled per modality pattern; SPMD shares
one program across batches so per-tile kinds are the union over batches).

Attention: keys on partitions (scores^T); softmax without max subtraction.
The softmax denominator is accumulated with DVE/GpSimd tensor adds over the
probability tiles and reduced with one ones-column matmul per (group, head) —
removing the per-key-tile denominator matmuls from the Tensor engine.
q_norm_w/k_norm_w fold into host-precomputed cos/sin tables; the RMS-norm
scale applies in the final rope multiply via a broadcast AP.  All rsqrt are
computed as exp(-0.5*ln(x)) so the whole kernel uses one ACT table set.

Groups are processed as [qkv tiles -> attention -> wo -> ReduceScatter]
so the collective stream starts early; the last group's RS splits into two
256-token chunks to shrink the end-of-kernel tail.  Final RMS-norms are
emitted deferred so engine queues never block on a collective.
"""
import sys

sys.path.insert(0, '/opt/trn_rl_repo')

from contextlib import ExitStack

import numpy as np
import ml_dtypes

import concourse.bass as bass
import concourse.tile as tile
from concourse import bacc, mybir
from concourse.bass import ts, ds, _add_dep_helper
from concourse.bass_utils import run_bass_kernel_spmd
from concourse.masks import make_identity

F32 = mybir.dt.float32
BF16 = mybir.dt.bfloat16

E = 2
HQ = 16
HK = 8
HD = 128
DIM = 2048
BS = 2
SEQ = 2048
EPS = 1e-6

N_CORES = 8
TP = 4                     # cores per batch group
HQC = HQ // TP             # 4 q heads per core
HKC = HK // TP             # 2 kv heads per core
DQ = HQC * HD              # 512 q cols per core
DKV = HKC * HD             # 256 k (and v) cols per core
NT = SEQ // 128            # 16 token tiles
KT = DIM // 128            # 16 contraction tiles
NG = 4                     # 512-token attention groups
GROUPS = [[0, 1, 2, 3], [4, 5, 6, 7]]
# RS chunk plan in tile units: (start_tile, n_tiles)
CHUNKS = ((0, 4), (4, 4), (8, 4), (12, 2), (14, 2))

_BUILD_CACHE = {}

MUL = mybir.AluOpType.mult
ADD = mybir.AluOpType.add
SUB = mybir.AluOpType.subtract
SHR = mybir.AluOpType.arith_shift_right
EXP = mybir.ActivationFunctionType.Exp
SQUARE = mybir.ActivationFunctionType.Square
I32 = mybir.dt.int32
RSQRT_MAGIC = 0x5F3759DF


def _chains_of(kinds):
    """kinds[T] in {0,1,2}; returns per-T list of experts to run."""
    return [[0, 1] if k == 2 else [k] for k in kinds]


def build_nc(has_qkw: bool, has_anw: bool, kinds: tuple):
    chains = _chains_of(kinds)
    nch = sum(len(c) for c in chains)
    duals = [T for T in range(NT) if kinds[T] == 2]
    dual_idx = {T: i for i, T in enumerate(duals)}
    nwod = max(1, len(duals))

    nc = bacc.Bacc("TRN2", target_bir_lowering=False, debug=False,
                   num_devices=N_CORES)

    xT = nc.dram_tensor("xT", [nch, 128, KT, 128], BF16, kind="ExternalInput")
    w0 = nc.dram_tensor("w0", [DIM, DQ + 2 * DKV], BF16, kind="ExternalInput")
    w1 = nc.dram_tensor("w1", [DIM, DQ + 2 * DKV], BF16, kind="ExternalInput")
    wo0 = nc.dram_tensor("wo0", [DQ, DIM], BF16, kind="ExternalInput")
    wo1 = nc.dram_tensor("wo1", [DQ, DIM], BF16, kind="ExternalInput")
    cosq = nc.dram_tensor("cosq", [SEQ, HD], BF16, kind="ExternalInput")
    sinq = nc.dram_tensor("sinq", [SEQ, HD], BF16, kind="ExternalInput")
    if has_qkw:
        cosk = nc.dram_tensor("cosk", [SEQ, HD], BF16, kind="ExternalInput")
        sink = nc.dram_tensor("sink", [SEQ, HD], BF16, kind="ExternalInput")
    dmin = nc.dram_tensor("dmin", [NT, 128, 512], BF16, kind="ExternalInput")
    wodm = nc.dram_tensor("wodm", [nwod, 2, 128, 128], BF16, kind="ExternalInput")
    if has_anw:
        anw0 = nc.dram_tensor("anw0", [1, DIM], F32, kind="ExternalInput")
        anwd = nc.dram_tensor("anwd", [1, DIM], F32, kind="ExternalInput")
        mfin = nc.dram_tensor("mfin", [128, len(CHUNKS)], F32, kind="ExternalInput")

    out_dram = nc.dram_tensor("out", [SEQ // 4, DIM], F32, kind="ExternalOutput")

    with tile.TileContext(nc) as tc:
        with ExitStack() as ctx:
            const = ctx.enter_context(tc.tile_pool(name="const", bufs=1))
            wpool = ctx.enter_context(tc.tile_pool(name="wpool", bufs=1))
            persist = ctx.enter_context(tc.tile_pool(name="persist", bufs=1))
            ropep = ctx.enter_context(tc.tile_pool(name="ropep", bufs=1))
            dram = ctx.enter_context(tc.tile_pool(name="dram", bufs=1, space="DRAM"))
            qtg = ctx.enter_context(tc.tile_pool(name="qtg", bufs=2))
            oftg = ctx.enter_context(tc.tile_pool(name="oftg", bufs=2))
            dmkp = ctx.enter_context(tc.tile_pool(name="dmkp", bufs=1))
            xpool = ctx.enter_context(tc.tile_pool(name="xpool", bufs=2))
            work = ctx.enter_context(tc.tile_pool(name="work", bufs=1))
            probs = ctx.enter_context(tc.tile_pool(name="probs", bufs=2))
            denp = ctx.enter_context(tc.tile_pool(name="denp", bufs=2))
            opool = ctx.enter_context(tc.tile_pool(name="opool", bufs=2))
            npool = ctx.enter_context(tc.tile_pool(name="npool", bufs=1))
            # PSUM: ps2 holds the 2-bank accumulators (qkv chains + score
            # tiles share one ring); otps the attention output accumulator;
            # ps1 the 1-bank transpose/wo/den tiles.  Total = 4+2+2 = 8 banks.
            ps2 = ctx.enter_context(tc.tile_pool(name="ps2", bufs=2, space="PSUM"))
            otps = ctx.enter_context(tc.tile_pool(name="otps", bufs=1, space="PSUM"))
            ps1 = ctx.enter_context(tc.tile_pool(name="ps1", bufs=2, space="PSUM"))

            # ---- constants ----
            identf = const.tile([128, 128], F32)
            make_identity(nc, identf[:])
            ident = const.tile([128, 128], BF16)
            nc.vector.tensor_copy(ident[:], identf[:])
            ones_col = const.tile([128, 1], BF16)
            nc.gpsimd.memset(ones_col[:], 1.0)
            ones_row = const.tile([1, 128], BF16)
            nc.gpsimd.memset(ones_row[:], 1.0)
            eps_1 = const.tile([128, 1], F32)
            nc.gpsimd.memset(eps_1[:], float(EPS))
            bias6 = const.tile([128, 6], F32)
            nc.gpsimd.memset(bias6[:, 0:4], float(128.0 * EPS))
            nc.gpsimd.memset(bias6[:, 4:6], float(EPS))
            magic = const.tile([128, 6], I32)
            nc.gpsimd.memset(magic[:], RSQRT_MAGIC)

            # ---- weights ----
            w0_sb = wpool.tile([128, KT, DQ + 2 * DKV], BF16)
            w1_sb = wpool.tile([128, KT, DQ + 2 * DKV], BF16)
            w0_r = w0.ap().rearrange("(k p) f -> p k f", p=128)
            w1_r = w1.ap().rearrange("(k p) f -> p k f", p=128)
            for k in range(KT):
                nc.gpsimd.dma_start(w0_sb[:, k, :], w0_r[:, k, :])
                nc.gpsimd.dma_start(w1_sb[:, k, :], w1_r[:, k, :])
            wo0_sb = wpool.tile([128, 4, DIM], BF16)
            nc.sync.dma_start(wo0_sb[:], wo0.ap().rearrange("(k p) f -> p k f", p=128))
            wo1_sb = wpool.tile([128, 4, DIM], BF16)
            nc.sync.dma_start(wo1_sb[:], wo1.ap().rearrange("(k p) f -> p k f", p=128))
            wodm_sb = wpool.tile([128, nwod, 2, 128], BF16)
            nc.sync.dma_start(wodm_sb[:], wodm.ap().rearrange("n e p c -> p n e c"))

            cq_sb = ropep.tile([128, NT, HD], BF16)
            nc.sync.dma_start(cq_sb[:], cosq.ap().rearrange("(t p) d -> p t d", p=128))
            sq_sb = ropep.tile([128, NT, HD], BF16)
            nc.sync.dma_start(sq_sb[:], sinq.ap().rearrange("(t p) d -> p t d", p=128))
            if has_qkw:
                ck_sb = ropep.tile([128, NT, HD], BF16)
                nc.sync.dma_start(ck_sb[:], cosk.ap().rearrange("(t p) d -> p t d", p=128))
                sk_sb = ropep.tile([128, NT, HD], BF16)
                nc.sync.dma_start(sk_sb[:], sink.ap().rearrange("(t p) d -> p t d", p=128))
            else:
                ck_sb, sk_sb = cq_sb, sq_sb

            if has_anw:
                anw0_sb = wpool.tile([1, DIM], F32)
                nc.sync.dma_start(anw0_sb[:], anw0[:, :])
                anwd_sb = wpool.tile([1, DIM], F32)
                nc.sync.dma_start(anwd_sb[:], anwd[:, :])
                anw0_b = wpool.tile([128, DIM], F32)
                nc.gpsimd.partition_broadcast(anw0_b[:], anw0_sb[:])
                anwd_b = wpool.tile([128, DIM], F32)
                nc.gpsimd.partition_broadcast(anwd_b[:], anwd_sb[:])
                mfin_sb = wpool.tile([128, len(CHUNKS)], F32)
                nc.sync.dma_start(mfin_sb[:], mfin[:, :])

            # persistent K^T / V for all groups
            KTb = persist.tile([128, HKC, SEQ], BF16)   # (hd, tok) per kv head
            Vb = persist.tile([128, NT, DKV], BF16)     # (tok, hd) natural

            chain_base = [sum(len(c) for c in chains[:T]) for T in range(NT)]
            pending_rs = []

            def rsqrt_dve(y, v, p, w):
                # y = v^-0.5 on DVE only (quake seed + 2 Newton steps).
                # y, v: f32 APs [p, w], may alias.
                it = work.tile([128, 6], I32, tag="rsq_i", name="rsq_i")[0:p, 0:w]
                t = work.tile([128, 6], F32, tag="rsq_t", name="rsq_t")[0:p, 0:w]
                h = work.tile([128, 6], F32, tag="rsq_h", name="rsq_h")[0:p, 0:w]
                nc.vector.tensor_scalar_mul(h, v, -0.5)
                nc.vector.tensor_scalar(it, v.bitcast(I32), 1, None, SHR)
                nc.vector.tensor_tensor(y.bitcast(I32), magic[0:p, 0:w], it, SUB)
                for _ in range(2):
                    nc.vector.tensor_tensor(t, y, y, MUL)
                    nc.vector.tensor_tensor(t, t, h, MUL)
                    nc.vector.scalar_tensor_tensor(
                        out=y, in0=t, scalar=1.5, in1=y, op0=ADD, op1=MUL)

            def do_final_norm(ci, rs_out, nrow, dep=None):
                sum_sb = npool.tile([128, DIM], BF16, tag="sum_sb")
                first = nc.sync.dma_start(sum_sb[0:nrow, :], rs_out[:])
                if dep is not None:
                    _add_dep_helper(first.ins, dep.ins, sync=False,
                                    reason="defer norm past next chunk")
                fin = npool.tile([128, DIM], F32, tag="fin")
                z = npool.tile([128, 1], F32, tag="z")
                nc.vector.scalar_tensor_tensor(
                    out=fin[0:nrow, :], in0=sum_sb[0:nrow, :], scalar=1.0,
                    in1=sum_sb[0:nrow, :], op0=MUL, op1=MUL, accum_out=z[0:nrow, :])
                rz = npool.tile([128, 1], F32, tag="rz")
                nc.vector.tensor_scalar(rz[0:nrow, :], z[0:nrow, :],
                                        1.0 / float(DIM), float(EPS), MUL, ADD)
                rsqrt_dve(rz[0:nrow, :], rz[0:nrow, :], nrow, 1)
                nc.scalar.mul(fin[0:nrow, :], sum_sb[0:nrow, :], rz[0:nrow, :])
                if has_anw:
                    anw_sel = npool.tile([128, DIM], F32, tag="anw_sel")
                    nc.vector.scalar_tensor_tensor(
                        out=anw_sel[0:nrow, :], in0=anwd_b[0:nrow, :],
                        scalar=mfin_sb[0:nrow, ci:ci + 1],
                        in1=anw0_b[0:nrow, :], op0=MUL, op1=ADD)
                    nc.vector.tensor_mul(fin[0:nrow, :], fin[0:nrow, :],
                                         anw_sel[0:nrow, :])
                row0 = sum(CHUNKS[i][1] * 32 for i in range(ci))
                nc.sync.dma_start(out_dram.ap()[row0:row0 + nrow, :], fin[0:nrow, :])

            for g in range(NG):
                # ---------------- phase 1: qkv for tiles of group g ----------
                QTg = qtg.tile([128, HKC, 1024], BF16, tag="qtg")
                for t in range(4):
                    T = 4 * g + t
                    pa = ps2.tile([128, 2, 512], F32, tag="acc")
                    for ci_, e in enumerate(chains[T]):
                        xt = xpool.tile([128, KT, 128], BF16, tag="xt")
                        nc.sync.dma_start(xt[:], xT.ap()[chain_base[T] + ci_])
                        w_sb = w1_sb if e == 1 else w0_sb
                        first = ci_ == 0
                        last = ci_ == len(chains[T]) - 1
                        for k in range(KT):
                            st = first and k == 0
                            sp = last and k == KT - 1
                            lhsT = xt[:, k, :]
                            nc.tensor.matmul(pa[:, 0, :], lhsT, w_sb[:, k, 0:512],
                                             start=st, stop=sp)
                            nc.tensor.matmul(pa[:, 1, :], lhsT, w_sb[:, k, 512:1024],
                                             start=st, stop=sp)

                    # V evict (natural layout)
                    nc.scalar.copy(Vb[:, T, :], pa[:, 1, 256:512])

                    # rms stats (ACT): sum of squares per head -> rsqrt
                    msq = work.tile([128, 6], F32, tag="msq")
                    scr = work.tile([128, 128], F32, tag="scr")
                    for h in range(HQC):
                        nc.scalar.activation(scr[:], pa[:, 0, ts(h, 128)],
                                             SQUARE, accum_out=msq[:, h:h + 1])
                    for h in range(HKC):
                        nc.scalar.activation(scr[:], pa[:, 1, ts(h, 128)],
                                             SQUARE, scale=float(128.0 ** -0.5),
                                             accum_out=msq[:, 4 + h:5 + h])
                    # q cols hold raw ssq (folds the 1/sqrt(HD) softmax scale
                    # into rs); k cols hold mean-square.  v = msq + bias
                    rs = work.tile([128, 6], F32, tag="rs")
                    nc.vector.tensor_tensor(rs[:], msq[:], bias6[:], ADD)
                    rsqrt_dve(rs[:], rs[:], 128, 6)

                    # batched rope; rms scale applied in final broadcast mul
                    def rope(src2, nh, cos_t, sin_t, rcols, dst):
                        w = nh * 128
                        t1 = work.tile([128, 512], F32, tag="rope_t1")
                        t2 = work.tile([128, 512], F32, tag="rope_t2")
                        sv = src2.rearrange("p (h d) -> p h d", h=nh)
                        cb = cos_t.unsqueeze(1).to_broadcast([128, nh, HD])
                        nc.vector.tensor_tensor(
                            out=t1[:, 0:w].rearrange("p (h d) -> p h d", h=nh),
                            in0=sv, in1=cb, op=MUL)
                        s0 = sin_t[:, 0:64].unsqueeze(1).to_broadcast([128, nh, 64])
                        s1 = sin_t[:, 64:128].unsqueeze(1).to_broadcast([128, nh, 64])
                        t2v = t2[:, 0:w].rearrange("p (h d) -> p h d", h=nh)
                        nc.vector.tensor_tensor(
                            out=t2v[:, :, 0:64], in0=sv[:, :, 64:128], in1=s0, op=MUL)
                        nc.vector.tensor_tensor(
                            out=t2v[:, :, 64:128], in0=sv[:, :, 0:64], in1=s1, op=MUL)
                        nc.vector.tensor_add(t1[:, 0:w], t1[:, 0:w], t2[:, 0:w])
                        rb = rcols.unsqueeze(2).to_broadcast([128, nh, HD])
                        nc.vector.tensor_tensor(
                            out=dst.rearrange("p (h d) -> p h d", h=nh),
                            in0=t1[:, 0:w].rearrange("p (h d) -> p h d", h=nh),
                            in1=rb, op=MUL)

                    q_rot = work.tile([128, 512], BF16, tag="q_rot")
                    rope(pa[:, 0, :], HQC, cq_sb[:, T, :], sq_sb[:, T, :],
                         rs[:, 0:4], q_rot[:])
                    k_rot = work.tile([128, 256], BF16, tag="k_rot")
                    rope(pa[:, 1, 0:256], HKC, ck_sb[:, T, :], sk_sb[:, T, :],
                         rs[:, 4:6], k_rot[:])

                    # transpose to (hd, tok); bf16, 4 per psum bank, one
                    # strided evict copy for q and one for k
                    tpq = ps1.tile([128, 4, 128], BF16, tag="tp", bufs=1)
                    for h in range(HQC):
                        nc.tensor.transpose(tpq[:, h, :], q_rot[:, ts(h, 128)],
                                            ident[:])
                    qtv = QTg[:].rearrange("p hp (dh c) -> p hp dh c", dh=2)
                    nc.vector.tensor_copy(
                        qtv[:, :, :, ts(t, 128)],
                        tpq[:].rearrange("p (hp dh) c -> p hp dh c", dh=2))
                    tpk = ps1.tile([128, 4, 128], BF16, tag="tp", bufs=1)
                    for h in range(HKC):
                        nc.tensor.transpose(tpk[:, h, :], k_rot[:, ts(h, 128)],
                                            ident[:])
                    nc.vector.tensor_copy(KTb[:, :, ts(T, 128)], tpk[:, 0:2, :])

                # ---------------- attention for group g ----------------
                dmk = dmkp.tile([128, 4, 512], BF16, tag="dmk")
                nc.gpsimd.dma_start(
                    dmk[:], dmin.ap()[ds(4 * g, 4)].rearrange("t p f -> p t f"))
                ofT = oftg.tile([128, HQC, 512], BF16, tag="oft")
                njt = 4 * (g + 1)
                for hp in range(HKC):
                    ot_ps = otps.tile([128, 2, 512], F32, tag="ot")
                    den_acc = denp.tile([128, 2, 512], F32, tag="dacc")
                    dve_den = (g + hp) % 2 == 0
                    for j in range(njt):
                        s_ps = ps2.tile([128, 2, 512], F32, tag="acc")
                        for dh in range(2):
                            nc.tensor.matmul(
                                s_ps[:, dh, :], KTb[:, hp, ts(j, 128)],
                                QTg[:, hp, ts(dh, 512)], start=True, stop=True)
                        p_t = probs.tile([128, 2, 512], BF16, tag="p")
                        nc.scalar.activation(p_t[:], s_ps[:], EXP)
                        if j >= 4 * g:
                            pm_t = probs.tile([128, 2, 512], BF16, tag="pm")
                            mb = dmk[:, j - 4 * g, :].unsqueeze(1).to_broadcast(
                                [128, 2, 512])
                            nc.vector.tensor_tensor(
                                out=pm_t[:], in0=p_t[:], in1=mb, op=MUL)
                            p_t = pm_t
                        eng = nc.vector if dve_den else nc.gpsimd
                        if j == 0:
                            eng.tensor_copy(den_acc[:], p_t[:])
                        else:
                            eng.tensor_add(den_acc[:], den_acc[:], p_t[:])
                        for dh in range(2):
                            nc.tensor.matmul(
                                ot_ps[:, dh, :], Vb[:, j, ts(hp, 128)],
                                p_t[:, dh, :], start=(j == 0), stop=(j == njt - 1))
                    # raw evict; denominator applied below
                    nc.vector.tensor_copy(ofT[:, 2 * hp:2 * hp + 2, :], ot_ps[:])
                    den16 = denp.tile([128, 2, 512], BF16, tag="d16", bufs=1)
                    nc.gpsimd.tensor_copy(den16[:], den_acc[:])
                    for dh in range(2):
                        dn_ps = ps1.tile([128, 512], F32, tag="wop", bufs=1)
                        nc.tensor.matmul(dn_ps[0:1, :], ones_col[:],
                                         den16[:, dh, :], start=True, stop=True)
                        den_r = work.tile([1, 512], F32, tag="den_r")
                        nc.vector.reciprocal_approx_fast(den_r[:], dn_ps[0:1, :])
                        den_rb = work.tile([1, 512], BF16, tag="den_rb")
                        nc.vector.tensor_copy(den_rb[:], den_r[:])
                        db_ps = ps1.tile([128, 512], F32, tag="wop", bufs=1)
                        nc.tensor.matmul(db_ps[:], ones_row[:], den_rb[:],
                                         start=True, stop=True)
                        nc.vector.tensor_mul(
                            ofT[:, 2 * hp + dh, :], ofT[:, 2 * hp + dh, :],
                            db_ps[:])

                # ---------------- wo + RS for chunks inside group g ---------
                for ci, (t0, ntl) in enumerate(CHUNKS):
                    if not (4 * g <= t0 and t0 + ntl <= 4 * g + 4):
                        continue
                    rs_in = dram.tile([ntl * 128, DIM], BF16, tag="rs_in", bufs=2)
                    last_rsin_dma = None
                    for u in range(ntl):
                        T = t0 + u
                        t = T - 4 * g
                        ch = chains[T]
                        srcs = []
                        if kinds[T] == 2:
                            di = dual_idx[T]
                            for e in (0, 1):
                                oz = opool.tile([128, HQC, 128], BF16,
                                                tag=f"oz{e}", bufs=1)
                                mb = wodm_sb[:, di, e, :].unsqueeze(1) \
                                    .to_broadcast([128, HQC, 128])
                                nc.vector.tensor_tensor(
                                    out=oz[:], in0=ofT[:, :, ts(t, 128)],
                                    in1=mb, op=MUL)
                                srcs.append(oz)
                        else:
                            srcs.append(None)
                        o_sb = opool.tile([128, DIM], BF16, tag="o_sb")
                        for n in range(4):
                            wo_ps = ps1.tile([128, 512], F32, tag="wop", bufs=1)
                            for ci_, e in enumerate(ch):
                                wo_sb = wo1_sb if e == 1 else wo0_sb
                                src = srcs[ci_]
                                for kk in range(4):
                                    lhsT = (ofT[:, kk, ts(t, 128)] if src is None
                                            else src[:, kk, :])
                                    nc.tensor.matmul(
                                        wo_ps[:], lhsT, wo_sb[:, kk, ts(n, 512)],
                                        start=(ci_ == 0 and kk == 0),
                                        stop=(ci_ == len(ch) - 1 and kk == 3))
                            if (u + n) % 2 == 0:
                                nc.vector.tensor_copy(o_sb[:, ts(n, 512)], wo_ps[:])
                            else:
                                nc.scalar.copy(o_sb[:, ts(n, 512)], wo_ps[:])
                        last_rsin_dma = nc.sync.dma_start(rs_in[ts(u, 128), :], o_sb[:])

                    rs_out = dram.tile([ntl * 32, DIM], BF16, tag="rs_out", bufs=2)
                    nc.gpsimd.collective_compute(
                        "ReduceScatter", mybir.AluOpType.add,
                        replica_groups=GROUPS,
                        ins=[rs_in.opt()], outs=[rs_out.opt()])
                    pending_rs.append((ci, rs_out, ntl * 32))
                    if len(pending_rs) > 1:
                        pci, prs, pnr = pending_rs.pop(0)
                        do_final_norm(pci, prs, pnr, dep=last_rsin_dma)

            for pci, prs, pnr in pending_rs:
                do_final_norm(pci, prs, pnr)

    nc.compile()
    return nc


def _plan(modality_ids):
    """Per-group stable modality sort; union tile kinds across batches."""
    mids = np.asarray(modality_ids).reshape(BS, SEQ)
    perms = np.empty((BS, SEQ), np.int64)   # permuted pos -> original token idx
    bkinds = np.empty((BS, NT), np.int64)
    for b in range(BS):
        for G in range(NG):
            mg = mids[b, 512 * G:512 * (G + 1)]
            i0 = np.where(mg == 0)[0]
            i1 = np.where(mg == 1)[0]
            perms[b, 512 * G:512 * (G + 1)] = 512 * G + np.concatenate([i0, i1])
            n0 = len(i0)
            for t in range(4):
                lo, hi = 128 * t, 128 * (t + 1)
                if hi <= n0:
                    bkinds[b, 4 * G + t] = 0
                elif lo >= n0:
                    bkinds[b, 4 * G + t] = 1
                else:
                    bkinds[b, 4 * G + t] = 2
    kinds = []
    for T in range(NT):
        k0, k1 = bkinds[0, T], bkinds[1, T]
        kinds.append(int(k0) if k0 == k1 else 2)
    return perms, tuple(kinds)


def _prep_inputs(x, freqs_cos, freqs_sin, wq, wk, wv, wo,
                 q_norm_w, k_norm_w, attn_norm_w, modality_ids,
                 has_qkw, has_anw, perms, kinds):
    """Build the 8 per-core input maps (numpy marshaling only)."""
    x = np.asarray(x, np.float32)
    freqs_cos = np.asarray(freqs_cos, np.float32)
    freqs_sin = np.asarray(freqs_sin, np.float32)
    wq = np.asarray(wq, np.float32)
    wk = np.asarray(wk, np.float32)
    wv = np.asarray(wv, np.float32)
    wo = np.asarray(wo, np.float32)
    mids = np.asarray(modality_ids).reshape(BS, SEQ)
    chains = _chains_of(kinds)
    duals = [T for T in range(NT) if kinds[T] == 2]
    nwod = max(1, len(duals))

    # de-interleave the hd dimension: [even dims, odd dims]
    perm_hd = np.concatenate([np.arange(0, HD, 2), np.arange(1, HD, 2)])

    def permute_heads(w, nh):
        w4 = w.reshape(E, DIM, nh, HD)
        return w4[:, :, :, perm_hd].reshape(E, DIM, nh * HD)

    wq_p = permute_heads(wq, HQ)
    wk_p = permute_heads(wk, HK)
    wv_p = permute_heads(wv, HK)
    wo4 = wo.reshape(E, HQ, HD, DIM)[:, :, perm_hd, :].reshape(E, HQ * HD, DIM)

    cosf = np.concatenate([freqs_cos, freqs_cos], axis=1)          # (SEQ, HD)
    sinf = np.concatenate([-freqs_sin, freqs_sin], axis=1)         # (SEQ, HD)
    qw = np.asarray(q_norm_w, np.float32)[:, perm_hd]              # (E, HD)
    kw = np.asarray(k_norm_w, np.float32)[:, perm_hd]

    in_maps = []
    for c in range(N_CORES):
        b, r = divmod(c, TP)
        P = perms[b]
        m = mids[b][P]                       # modality per permuted position
        qs = slice(r * DQ, (r + 1) * DQ)
        ks = slice(r * DKV, (r + 1) * DKV)
        w0c = np.concatenate([wq_p[0][:, qs], wk_p[0][:, ks], wv_p[0][:, ks]], axis=1)
        w1c = np.concatenate([wq_p[1][:, qs], wk_p[1][:, ks], wv_p[1][:, ks]], axis=1)

        # x^T chain tiles with per-expert zeroing on dual tiles
        xTb = x[b].T[:, P]                   # (dim, seq) permuted
        xt_list = []
        for T in range(NT):
            tile_x = xTb[:, 128 * T:128 * (T + 1)]   # (dim, 128)
            mt = m[128 * T:128 * (T + 1)]
            for e in chains[T]:
                if kinds[T] == 2:
                    tx = tile_x * (mt == e)[None, :]
                else:
                    tx = tile_x
                xt_list.append(tx.reshape(KT, 128, 128))
        xTc = np.stack(xt_list)              # (nch, KT, 128dim, 128tok)
        xTc = np.ascontiguousarray(xTc.transpose(0, 2, 1, 3))

        # per-token folded cos/sin (q and k norm weights)
        cq = cosf[P] * qw[m]
        sq = sinf[P] * qw[m]

        # in-group causal masks for the permuted order
        pos = (P % 512)
        dmv = np.zeros((NT, 128, 512), np.float32)
        for j in range(NT):
            gj = j // 4
            kpos = pos[128 * j:128 * (j + 1)]
            qpos = pos[512 * gj:512 * (gj + 1)]
            dmv[j] = (kpos[:, None] <= qpos[None, :])

        # wo dual masks (pre-broadcast rows)
        wodmv = np.zeros((nwod, 2, 128, 128), np.float32)
        for i, T in enumerate(duals):
            mt = m[128 * T:128 * (T + 1)]
            for e in (0, 1):
                wodmv[i, e] = np.tile((mt == e)[None, :].astype(np.float32),
                                      (128, 1))

        im = {
            "xT": xTc.astype(ml_dtypes.bfloat16),
            "w0": w0c.astype(ml_dtypes.bfloat16),
            "w1": w1c.astype(ml_dtypes.bfloat16),
            "wo0": wo4[0][r * DQ:(r + 1) * DQ, :].astype(ml_dtypes.bfloat16),
            "wo1": wo4[1][r * DQ:(r + 1) * DQ, :].astype(ml_dtypes.bfloat16),
            "cosq": np.ascontiguousarray(cq).astype(ml_dtypes.bfloat16),
            "sinq": np.ascontiguousarray(sq).astype(ml_dtypes.bfloat16),
            "dmin": dmv.astype(ml_dtypes.bfloat16),
            "wodm": wodmv.astype(ml_dtypes.bfloat16),
        }
        if has_qkw:
            im["cosk"] = np.ascontiguousarray(cosf[P] * kw[m]).astype(ml_dtypes.bfloat16)
            im["sink"] = np.ascontiguousarray(sinf[P] * kw[m]).astype(ml_dtypes.bfloat16)
        if has_anw:
            aw = np.asarray(attn_norm_w, np.float32)
            im["anw0"] = np.ascontiguousarray(aw[0:1])
            im["anwd"] = (aw[1] - aw[0]).reshape(1, DIM).copy()
            mf = np.zeros((128, len(CHUNKS)), np.float32)
            for ci, (t0, ntl) in enumerate(CHUNKS):
                nrow = ntl * 32
                t0tok = t0 * 128
                mf[0:nrow, ci] = m[t0tok + nrow * r: t0tok + nrow * (r + 1)]
            im["mfin"] = mf
        in_maps.append(im)
    return in_maps


def kernel(**inputs):
    q_norm_w = np.asarray(inputs["q_norm_w"], np.float32)
    k_norm_w = np.asarray(inputs["k_norm_w"], np.float32)
    attn_norm_w = np.asarray(inputs["attn_norm_w"], np.float32)
    has_qkw = not np.array_equal(q_norm_w, k_norm_w)
    has_anw = not np.all(attn_norm_w == 1.0)

    perms, kinds = _plan(inputs["modality_ids"])
    key = (has_qkw, has_anw, kinds)
    if key not in _BUILD_CACHE:
        _BUILD_CACHE[key] = build_nc(has_qkw, has_anw, kinds)
    nc = _BUILD_CACHE[key]

    in_maps = _prep_inputs(
        inputs["x"], inputs["freqs_cos"], inputs["freqs_sin"],
        inputs["wq"], inputs["wk"], inputs["wv"], inputs["wo"],
        q_norm_w, k_norm_w, attn_norm_w, inputs["modality_ids"],
        has_qkw, has_anw, perms, kinds)

    res = run_bass_kernel_spmd(nc, in_maps, core_ids=list(range(N_CORES)))

    out = np.empty((BS, SEQ, DIM), np.float32)
    for c in range(N_CORES):
        b, r = divmod(c, TP)
        P = perms[b]
        oc = res.results[c]["out"]          # (SEQ//4, DIM), permuted rows
        row = 0
        for ci, (t0, ntl) in enumerate(CHUNKS):
            nrow = ntl * 32
            t0tok = t0 * 128
            sel = P[t0tok + nrow * r: t0tok + nrow * (r + 1)]
            out[b, sel, :] = oc[row:row + nrow, :]
            row += nrow
    return out


# revision 11
# speedup vs baseline: 1.0444x; 1.0444x over previous
"""ModalityUntiedAttention on 8 TRN2 NeuronCores (Bass/Tile) — v2.

Sharding: data-parallel over batch (cores 0-3 -> batch 0, cores 4-7 -> batch 1),
tensor-parallel over heads within each 4-core group (4 q heads + 2 kv heads per
core).

Expert (modality) routing: tokens are sorted by modality WITHIN each 512-token
attention group (host-side permutation), then routed at 128-token-tile
granularity with ZERO on-device select ops: a tile that is pure in both
batches runs one expert's weight chain; a tile that straddles the modality
boundary in either batch runs BOTH experts' chains accumulating into the same
PSUM, with the host zeroing each chain's x columns for tokens that belong to
the other expert (exact routing, compiled per modality pattern; SPMD shares
one program across batches so per-tile kinds are the union over batches).

Attention: keys on partitions (scores^T); softmax without max subtraction.
The softmax denominator is accumulated with DVE/GpSimd tensor adds over the
probability tiles and reduced with one ones-column matmul per (group, head) —
removing the per-key-tile denominator matmuls from the Tensor engine.
q_norm_w/k_norm_w fold into host-precomputed cos/sin tables; the RMS-norm
scale applies in the final rope multiply via a broadcast AP.  All rsqrt are
computed as exp(-0.5*ln(x)) so the whole kernel uses one ACT table set.

Groups are processed as [qkv tiles -> attention -> wo -> ReduceScatter]
so the collective stream starts early; the last group's RS splits into two
256-token chunks to shrink the end-of-kernel tail.  Final RMS-norms are
emitted deferred so engine queues never block on a collective.
"""
import sys

sys.path.insert(0, '/opt/trn_rl_repo')

from contextlib import ExitStack

import numpy as np
import ml_dtypes

import concourse.bass as bass
import concourse.tile as tile
from concourse import bacc, mybir
from concourse.bass import ts, ds, _add_dep_helper
from concourse.bass_utils import run_bass_kernel_spmd
from concourse.masks import make_identity

F32 = mybir.dt.float32
BF16 = mybir.dt.bfloat16

E = 2
HQ = 16
HK = 8
HD = 128
DIM = 2048
BS = 2
SEQ = 2048
EPS = 1e-6

N_CORES = 8
TP = 4                     # cores per batch group
HQC = HQ // TP             # 4 q heads per core
HKC = HK // TP             # 2 kv heads per core
DQ = HQC * HD              # 512 q cols per core
DKV = HKC * HD             # 256 k (and v) cols per core
NT = SEQ // 128            # 16 token tiles
KT = DIM // 128            # 16 contraction tiles
NG = 4                     # 512-token attention groups
GROUPS = [[0, 1, 2, 3], [4, 5, 6, 7]]
# RS chunk plan in tile units: (start_tile, n_tiles)
CHUNKS = ((0, 4), (4, 4), (8, 4), (12, 2), (14, 2))

_BUILD_CACHE = {}

MUL = mybir.AluOpType.mult
ADD = mybir.AluOpType.add
SUB = mybir.AluOpType.subtract
SHR = mybir.AluOpType.arith_shift_right
EXP = mybir.ActivationFunctionType.Exp
SQUARE = mybir.ActivationFunctionType.Square
I32 = mybir.dt.int32
RSQRT_MAGIC = 0x5F3759DF


def _chains_of(kinds):
    """kinds[T] in {0,1,2}; returns per-T list of experts to run."""
    return [[0, 1] if k == 2 else [k] for k in kinds]


def build_nc(has_qkw: bool, has_anw: bool, kinds: tuple):
    chains = _chains_of(kinds)
    nch = sum(len(c) for c in chains)
    duals = [T for T in range(NT) if kinds[T] == 2]
    dual_idx = {T: i for i, T in enumerate(duals)}
    nwod = max(1, len(duals))

    nc = bacc.Bacc("TRN2", target_bir_lowering=False, debug=False,
                   num_devices=N_CORES)

    xT = nc.dram_tensor("xT", [nch, 128, KT, 128], BF16, kind="ExternalInput")
    w0 = nc.dram_tensor("w0", [DIM, DQ + 2 * DKV], BF16, kind="ExternalInput")
    w1 = nc.dram_tensor("w1", [DIM, DQ + 2 * DKV], BF16, kind="ExternalInput")
    wo0 = nc.dram_tensor("wo0", [DQ, DIM], BF16, kind="ExternalInput")
    wo1 = nc.dram_tensor("wo1", [DQ, DIM], BF16, kind="ExternalInput")
    cosq = nc.dram_tensor("cosq", [SEQ, HD], BF16, kind="ExternalInput")
    sinq = nc.dram_tensor("sinq", [SEQ, HD], BF16, kind="ExternalInput")
    if has_qkw:
        cosk = nc.dram_tensor("cosk", [SEQ, HD], BF16, kind="ExternalInput")
        sink = nc.dram_tensor("sink", [SEQ, HD], BF16, kind="ExternalInput")
    dmin = nc.dram_tensor("dmin", [NT, 128, 512], BF16, kind="ExternalInput")
    wodm = nc.dram_tensor("wodm", [nwod, 2, 128, 128], BF16, kind="ExternalInput")
    if has_anw:
        anw0 = nc.dram_tensor("anw0", [1, DIM], F32, kind="ExternalInput")
        anwd = nc.dram_tensor("anwd", [1, DIM], F32, kind="ExternalInput")
        mfin = nc.dram_tensor("mfin", [128, len(CHUNKS)], F32, kind="ExternalInput")

    out_dram = nc.dram_tensor("out", [SEQ // 4, DIM], F32, kind="ExternalOutput")

    with tile.TileContext(nc) as tc:
        with ExitStack() as ctx:
            const = ctx.enter_context(tc.tile_pool(name="const", bufs=1))
            wpool = ctx.enter_context(tc.tile_pool(name="wpool", bufs=1))
            persist = ctx.enter_context(tc.tile_pool(name="persist", bufs=1))
            ropep = ctx.enter_context(tc.tile_pool(name="ropep", bufs=1))
            dram = ctx.enter_context(tc.tile_pool(name="dram", bufs=1, space="DRAM"))
            qtg = ctx.enter_context(tc.tile_pool(name="qtg", bufs=2))
            oftg = ctx.enter_context(tc.tile_pool(name="oftg", bufs=2))
            dmkp = ctx.enter_context(tc.tile_pool(name="dmkp", bufs=1))
            xpool = ctx.enter_context(tc.tile_pool(name="xpool", bufs=2))
            work = ctx.enter_context(tc.tile_pool(name="work", bufs=1))
            probs = ctx.enter_context(tc.tile_pool(name="probs", bufs=2))
            denp = ctx.enter_context(tc.tile_pool(name="denp", bufs=2))
            opool = ctx.enter_context(tc.tile_pool(name="opool", bufs=2))
            npool = ctx.enter_context(tc.tile_pool(name="npool", bufs=1))
            # PSUM: ps2 holds the 2-bank accumulators (qkv chains + score
            # tiles share one ring); otps the attention output accumulator;
            # ps1 the 1-bank transpose/wo/den tiles.  Total = 4+2+2 = 8 banks.
            ps2 = ctx.enter_context(tc.tile_pool(name="ps2", bufs=2, space="PSUM"))
            otps = ctx.enter_context(tc.tile_pool(name="otps", bufs=1, space="PSUM"))
            ps1 = ctx.enter_context(tc.tile_pool(name="ps1", bufs=2, space="PSUM"))

            # ---- constants ----
            identf = const.tile([128, 128], F32)
            make_identity(nc, identf[:])
            ident = const.tile([128, 128], BF16)
            nc.vector.tensor_copy(ident[:], identf[:])
            ones_col = const.tile([128, 1], BF16)
            nc.gpsimd.memset(ones_col[:], 1.0)
            ones_row = const.tile([1, 128], BF16)
            nc.gpsimd.memset(ones_row[:], 1.0)
            eps_1 = const.tile([128, 1], F32)
            nc.gpsimd.memset(eps_1[:], float(EPS))
            bias6 = const.tile([128, 6], F32)
            nc.gpsimd.memset(bias6[:, 0:4], float(128.0 * EPS))
            nc.gpsimd.memset(bias6[:, 4:6], float(EPS))
            magic = const.tile([128, 6], I32)
            nc.gpsimd.memset(magic[:], RSQRT_MAGIC)

            # ---- weights ----
            w0_sb = wpool.tile([128, KT, DQ + 2 * DKV], BF16)
            w1_sb = wpool.tile([128, KT, DQ + 2 * DKV], BF16)
            w0_r = w0.ap().rearrange("(k p) f -> p k f", p=128)
            w1_r = w1.ap().rearrange("(k p) f -> p k f", p=128)
            for k in range(KT):
                nc.gpsimd.dma_start(w0_sb[:, k, :], w0_r[:, k, :])
                nc.gpsimd.dma_start(w1_sb[:, k, :], w1_r[:, k, :])
            wo0_sb = wpool.tile([128, 4, DIM], BF16)
            nc.gpsimd.dma_start(wo0_sb[:], wo0.ap().rearrange("(k p) f -> p k f", p=128))
            wo1_sb = wpool.tile([128, 4, DIM], BF16)
            nc.gpsimd.dma_start(wo1_sb[:], wo1.ap().rearrange("(k p) f -> p k f", p=128))
            wodm_sb = wpool.tile([128, nwod, 2, 128], BF16)
            nc.gpsimd.dma_start(wodm_sb[:], wodm.ap().rearrange("n e p c -> p n e c"))

            cq_sb = ropep.tile([128, NT, HD], BF16)
            nc.sync.dma_start(cq_sb[:], cosq.ap().rearrange("(t p) d -> p t d", p=128))
            sq_sb = ropep.tile([128, NT, HD], BF16)
            nc.sync.dma_start(sq_sb[:], sinq.ap().rearrange("(t p) d -> p t d", p=128))
            if has_qkw:
                ck_sb = ropep.tile([128, NT, HD], BF16)
                nc.sync.dma_start(ck_sb[:], cosk.ap().rearrange("(t p) d -> p t d", p=128))
                sk_sb = ropep.tile([128, NT, HD], BF16)
                nc.sync.dma_start(sk_sb[:], sink.ap().rearrange("(t p) d -> p t d", p=128))
            else:
                ck_sb, sk_sb = cq_sb, sq_sb

            if has_anw:
                anw0_sb = wpool.tile([1, DIM], F32)
                nc.sync.dma_start(anw0_sb[:], anw0[:, :])
                anwd_sb = wpool.tile([1, DIM], F32)
                nc.sync.dma_start(anwd_sb[:], anwd[:, :])
                anw0_b = wpool.tile([128, DIM], F32)
                nc.gpsimd.partition_broadcast(anw0_b[:], anw0_sb[:])
                anwd_b = wpool.tile([128, DIM], F32)
                nc.gpsimd.partition_broadcast(anwd_b[:], anwd_sb[:])
                mfin_sb = wpool.tile([128, len(CHUNKS)], F32)
                nc.sync.dma_start(mfin_sb[:], mfin[:, :])

            # persistent K^T / V for all groups
            KTb = persist.tile([128, HKC, SEQ], BF16)   # (hd, tok) per kv head
            Vb = persist.tile([128, NT, DKV], BF16)     # (tok, hd) natural

            chain_base = [sum(len(c) for c in chains[:T]) for T in range(NT)]
            pending_rs = []

            def rsqrt_dve(y, v, p, w):
                # y = v^-0.5 on DVE only (quake seed + 2 Newton steps).
                # y, v: f32 APs [p, w], may alias.
                it = work.tile([128, 6], I32, tag="rsq_i", name="rsq_i")[0:p, 0:w]
                t = work.tile([128, 6], F32, tag="rsq_t", name="rsq_t")[0:p, 0:w]
                h = work.tile([128, 6], F32, tag="rsq_h", name="rsq_h")[0:p, 0:w]
                nc.vector.tensor_scalar_mul(h, v, -0.5)
                nc.vector.tensor_scalar(it, v.bitcast(I32), 1, None, SHR)
                nc.vector.tensor_tensor(y.bitcast(I32), magic[0:p, 0:w], it, SUB)
                for _ in range(2):
                    nc.vector.tensor_tensor(t, y, y, MUL)
                    nc.vector.tensor_tensor(t, t, h, MUL)
                    nc.vector.scalar_tensor_tensor(
                        out=y, in0=t, scalar=1.5, in1=y, op0=ADD, op1=MUL)

            def do_final_norm(ci, rs_out, nrow, dep=None):
                sum_sb = npool.tile([128, DIM], BF16, tag="sum_sb")
                first = nc.gpsimd.dma_start(sum_sb[0:nrow, :], rs_out[:])
                if dep is not None:
                    _add_dep_helper(first.ins, dep.ins, sync=False,
                                    reason="defer norm past next chunk")
                fin = npool.tile([128, DIM], F32, tag="fin")
                z = npool.tile([128, 1], F32, tag="z")
                nc.vector.scalar_tensor_tensor(
                    out=fin[0:nrow, :], in0=sum_sb[0:nrow, :], scalar=1.0,
                    in1=sum_sb[0:nrow, :], op0=MUL, op1=MUL, accum_out=z[0:nrow, :])
                rz = npool.tile([128, 1], F32, tag="rz")
                nc.vector.tensor_scalar(rz[0:nrow, :], z[0:nrow, :],
                                        1.0 / float(DIM), float(EPS), MUL, ADD)
                rsqrt_dve(rz[0:nrow, :], rz[0:nrow, :], nrow, 1)
                nc.scalar.mul(fin[0:nrow, :], sum_sb[0:nrow, :], rz[0:nrow, :])
                if has_anw:
                    anw_sel = npool.tile([128, DIM], F32, tag="anw_sel")
                    nc.vector.scalar_tensor_tensor(
                        out=anw_sel[0:nrow, :], in0=anwd_b[0:nrow, :],
                        scalar=mfin_sb[0:nrow, ci:ci + 1],
                        in1=anw0_b[0:nrow, :], op0=MUL, op1=ADD)
                    nc.vector.tensor_mul(fin[0:nrow, :], fin[0:nrow, :],
                                         anw_sel[0:nrow, :])
                row0 = sum(CHUNKS[i][1] * 32 for i in range(ci))
                nc.gpsimd.dma_start(out_dram.ap()[row0:row0 + nrow, :], fin[0:nrow, :])

            for g in range(NG):
                # ---------------- phase 1: qkv for tiles of group g ----------
                QTg = qtg.tile([128, HKC, 1024], BF16, tag="qtg")
                for t in range(4):
                    T = 4 * g + t
                    pa = ps2.tile([128, 2, 512], F32, tag="acc")
                    for ci_, e in enumerate(chains[T]):
                        xt = xpool.tile([128, KT, 128], BF16, tag="xt")
                        nc.sync.dma_start(xt[:], xT.ap()[chain_base[T] + ci_])
                        w_sb = w1_sb if e == 1 else w0_sb
                        first = ci_ == 0
                        last = ci_ == len(chains[T]) - 1
                        for k in range(KT):
                            st = first and k == 0
                            sp = last and k == KT - 1
                            lhsT = xt[:, k, :]
                            nc.tensor.matmul(pa[:, 0, :], lhsT, w_sb[:, k, 0:512],
                                             start=st, stop=sp)
                            nc.tensor.matmul(pa[:, 1, :], lhsT, w_sb[:, k, 512:1024],
                                             start=st, stop=sp)

                    # V evict (natural layout)
                    nc.scalar.copy(Vb[:, T, :], pa[:, 1, 256:512])
                    # single bf16 staging copy of q|k; rope + rms read SBUF
                    qk_sb = work.tile([128, 768], BF16, tag="qk_sb", bufs=2)
                    paf = pa[:].rearrange("p a b -> p (a b)")
                    nc.vector.tensor_copy(qk_sb[:], paf[:, 0:768])

                    # rms stats (DVE): sum of squares per head -> rsqrt
                    msq = work.tile([128, 6], F32, tag="msq")
                    scr = work.tile([128, 128], F32, tag="scr")
                    for h in range(6):
                        nc.vector.scalar_tensor_tensor(
                            out=scr[:], in0=qk_sb[:, ts(h, 128)],
                            scalar=1.0 if h < 4 else 1.0 / 128.0,
                            in1=qk_sb[:, ts(h, 128)], op0=MUL, op1=MUL,
                            accum_out=msq[:, h:h + 1])
                    # q cols hold raw ssq (folds the 1/sqrt(HD) softmax scale
                    # into rs); k cols hold mean-square.  v = msq + bias
                    rs = work.tile([128, 6], F32, tag="rs")
                    nc.vector.tensor_tensor(rs[:], msq[:], bias6[:], ADD)
                    rsqrt_dve(rs[:], rs[:], 128, 6)

                    # batched rope; rms scale applied in final broadcast mul
                    def rope(src2, nh, cos_t, sin_t, rcols, dst):
                        w = nh * 128
                        t1 = work.tile([128, 512], BF16, tag="rope_t1")
                        t2 = work.tile([128, 512], BF16, tag="rope_t2")
                        sv = src2.rearrange("p (h d) -> p h d", h=nh)
                        cb = cos_t.unsqueeze(1).to_broadcast([128, nh, HD])
                        nc.vector.tensor_tensor(
                            out=t1[:, 0:w].rearrange("p (h d) -> p h d", h=nh),
                            in0=sv, in1=cb, op=MUL)
                        s0 = sin_t[:, 0:64].unsqueeze(1).to_broadcast([128, nh, 64])
                        s1 = sin_t[:, 64:128].unsqueeze(1).to_broadcast([128, nh, 64])
                        t2v = t2[:, 0:w].rearrange("p (h d) -> p h d", h=nh)
                        nc.vector.tensor_tensor(
                            out=t2v[:, :, 0:64], in0=sv[:, :, 64:128], in1=s0, op=MUL)
                        nc.vector.tensor_tensor(
                            out=t2v[:, :, 64:128], in0=sv[:, :, 0:64], in1=s1, op=MUL)
                        nc.vector.tensor_add(t1[:, 0:w], t1[:, 0:w], t2[:, 0:w])
                        rb = rcols.unsqueeze(2).to_broadcast([128, nh, HD])
                        nc.vector.tensor_tensor(
                            out=dst.rearrange("p (h d) -> p h d", h=nh),
                            in0=t1[:, 0:w].rearrange("p (h d) -> p h d", h=nh),
                            in1=rb, op=MUL)

                    q_rot = work.tile([128, 512], BF16, tag="q_rot")
                    rope(qk_sb[:, 0:512], HQC, cq_sb[:, T, :], sq_sb[:, T, :],
                         rs[:, 0:4], q_rot[:])
                    k_rot = work.tile([128, 256], BF16, tag="k_rot")
                    rope(qk_sb[:, 512:768], HKC, ck_sb[:, T, :], sk_sb[:, T, :],
                         rs[:, 4:6], k_rot[:])

                    # transpose to (hd, tok); bf16, 4 per psum bank, one
                    # strided evict copy for q and one for k
                    tpq = ps1.tile([128, 4, 128], BF16, tag="tp", bufs=1)
                    for h in range(HQC):
                        nc.tensor.transpose(tpq[:, h, :], q_rot[:, ts(h, 128)],
                                            ident[:])
                    qtv = QTg[:].rearrange("p hp (dh c) -> p hp dh c", dh=2)
                    nc.scalar.copy(
                        qtv[:, :, :, ts(t, 128)],
                        tpq[:].rearrange("p (hp dh) c -> p hp dh c", dh=2))
                    tpk = ps1.tile([128, 4, 128], BF16, tag="tp", bufs=1)
                    for h in range(HKC):
                        nc.tensor.transpose(tpk[:, h, :], k_rot[:, ts(h, 128)],
                                            ident[:])
                    nc.vector.tensor_copy(KTb[:, :, ts(T, 128)], tpk[:, 0:2, :])

                # ---------------- attention for group g ----------------
                dmk = dmkp.tile([128, 4, 512], BF16, tag="dmk")
                nc.gpsimd.dma_start(
                    dmk[:], dmin.ap()[ds(4 * g, 4)].rearrange("t p f -> p t f"))
                ofT = oftg.tile([128, HQC, 512], BF16, tag="oft")
                njt = 4 * (g + 1)
                for hp in range(HKC):
                    ot_ps = otps.tile([128, 2, 512], F32, tag="ot")
                    den_acc = denp.tile([128, 2, 512], F32, tag="dacc")
                    for j in range(njt):
                        s_ps = ps2.tile([128, 2, 512], F32, tag="acc")
                        for dh in range(2):
                            nc.tensor.matmul(
                                s_ps[:, dh, :], KTb[:, hp, ts(j, 128)],
                                QTg[:, hp, ts(dh, 512)], start=True, stop=True)
                        p_t = probs.tile([128, 2, 512], BF16, tag="p")
                        nc.scalar.activation(p_t[:], s_ps[:], EXP)
                        if j >= 4 * g:
                            pm_t = probs.tile([128, 2, 512], BF16, tag="pm")
                            mb = dmk[:, j - 4 * g, :].unsqueeze(1).to_broadcast(
                                [128, 2, 512])
                            nc.vector.tensor_tensor(
                                out=pm_t[:], in0=p_t[:], in1=mb, op=MUL)
                            p_t = pm_t
                        if j == 0:
                            nc.vector.tensor_copy(den_acc[:], p_t[:])
                        else:
                            nc.vector.tensor_add(den_acc[:], den_acc[:], p_t[:])
                        for dh in range(2):
                            nc.tensor.matmul(
                                ot_ps[:, dh, :], Vb[:, j, ts(hp, 128)],
                                p_t[:, dh, :], start=(j == 0), stop=(j == njt - 1))
                    # raw evict; denominator applied below
                    nc.vector.tensor_copy(ofT[:, 2 * hp:2 * hp + 2, :], ot_ps[:])
                    den16 = denp.tile([128, 2, 512], BF16, tag="d16", bufs=1)
                    nc.vector.tensor_copy(den16[:], den_acc[:])
                    for dh in range(2):
                        dn_ps = ps1.tile([128, 512], F32, tag="wop", bufs=1)
                        nc.tensor.matmul(dn_ps[0:1, :], ones_col[:],
                                         den16[:, dh, :], start=True, stop=True)
                        den_r = work.tile([1, 512], F32, tag="den_r")
                        nc.vector.reciprocal_approx_fast(den_r[:], dn_ps[0:1, :])
                        den_rb = work.tile([1, 512], BF16, tag="den_rb")
                        nc.vector.tensor_copy(den_rb[:], den_r[:])
                        db_ps = ps1.tile([128, 512], F32, tag="wop", bufs=1)
                        nc.tensor.matmul(db_ps[:], ones_row[:], den_rb[:],
                                         start=True, stop=True)
                        nc.vector.tensor_mul(
                            ofT[:, 2 * hp + dh, :], ofT[:, 2 * hp + dh, :],
                            db_ps[:])

                # ---------------- wo + RS for chunks inside group g ---------
                for ci, (t0, ntl) in enumerate(CHUNKS):
                    if not (4 * g <= t0 and t0 + ntl <= 4 * g + 4):
                        continue
                    rs_in = dram.tile([ntl * 128, DIM], BF16, tag="rs_in", bufs=2)
                    last_rsin_dma = None
                    for u in range(ntl):
                        T = t0 + u
                        t = T - 4 * g
                        ch = chains[T]
                        srcs = []
                        if kinds[T] == 2:
                            di = dual_idx[T]
                            for e in (0, 1):
                                oz = opool.tile([128, HQC, 128], BF16,
                                                tag=f"oz{e}", bufs=1)
                                mb = wodm_sb[:, di, e, :].unsqueeze(1) \
                                    .to_broadcast([128, HQC, 128])
                                nc.vector.tensor_tensor(
                                    out=oz[:], in0=ofT[:, :, ts(t, 128)],
                                    in1=mb, op=MUL)
                                srcs.append(oz)
                        else:
                            srcs.append(None)
                        o_sb = opool.tile([128, DIM], BF16, tag="o_sb")
                        for n in range(4):
                            wo_ps = ps1.tile([128, 512], F32, tag="wop", bufs=1)
                            for ci_, e in enumerate(ch):
                                wo_sb = wo1_sb if e == 1 else wo0_sb
                                src = srcs[ci_]
                                for kk in range(4):
                                    lhsT = (ofT[:, kk, ts(t, 128)] if src is None
                                            else src[:, kk, :])
                                    nc.tensor.matmul(
                                        wo_ps[:], lhsT, wo_sb[:, kk, ts(n, 512)],
                                        start=(ci_ == 0 and kk == 0),
                                        stop=(ci_ == len(ch) - 1 and kk == 3))
                            if (u + n) % 2 == 0:
                                nc.vector.tensor_copy(o_sb[:, ts(n, 512)], wo_ps[:])
                            else:
                                nc.scalar.copy(o_sb[:, ts(n, 512)], wo_ps[:])
                        last_rsin_dma = nc.scalar.dma_start(rs_in[ts(u, 128), :], o_sb[:])

                    rs_out = dram.tile([ntl * 32, DIM], BF16, tag="rs_out", bufs=2)
                    nc.gpsimd.collective_compute(
                        "ReduceScatter", mybir.AluOpType.add,
                        replica_groups=GROUPS,
                        ins=[rs_in.opt()], outs=[rs_out.opt()])
                    pending_rs.append((ci, rs_out, ntl * 32))
                    if len(pending_rs) > 1:
                        pci, prs, pnr = pending_rs.pop(0)
                        do_final_norm(pci, prs, pnr, dep=last_rsin_dma)

            for pci, prs, pnr in pending_rs:
                do_final_norm(pci, prs, pnr)

    nc.compile()
    return nc


def _plan(modality_ids):
    """Per-group stable modality sort; union tile kinds across batches."""
    mids = np.asarray(modality_ids).reshape(BS, SEQ)
    perms = np.empty((BS, SEQ), np.int64)   # permuted pos -> original token idx
    bkinds = np.empty((BS, NT), np.int64)
    for b in range(BS):
        for G in range(NG):
            mg = mids[b, 512 * G:512 * (G + 1)]
            i0 = np.where(mg == 0)[0]
            i1 = np.where(mg == 1)[0]
            perms[b, 512 * G:512 * (G + 1)] = 512 * G + np.concatenate([i0, i1])
            n0 = len(i0)
            for t in range(4):
                lo, hi = 128 * t, 128 * (t + 1)
                if hi <= n0:
                    bkinds[b, 4 * G + t] = 0
                elif lo >= n0:
                    bkinds[b, 4 * G + t] = 1
                else:
                    bkinds[b, 4 * G + t] = 2
    kinds = []
    for T in range(NT):
        k0, k1 = bkinds[0, T], bkinds[1, T]
        kinds.append(int(k0) if k0 == k1 else 2)
    return perms, tuple(kinds)


def _prep_inputs(x, freqs_cos, freqs_sin, wq, wk, wv, wo,
                 q_norm_w, k_norm_w, attn_norm_w, modality_ids,
                 has_qkw, has_anw, perms, kinds):
    """Build the 8 per-core input maps (numpy marshaling only)."""
    x = np.asarray(x, np.float32)
    freqs_cos = np.asarray(freqs_cos, np.float32)
    freqs_sin = np.asarray(freqs_sin, np.float32)
    wq = np.asarray(wq, np.float32)
    wk = np.asarray(wk, np.float32)
    wv = np.asarray(wv, np.float32)
    wo = np.asarray(wo, np.float32)
    mids = np.asarray(modality_ids).reshape(BS, SEQ)
    chains = _chains_of(kinds)
    duals = [T for T in range(NT) if kinds[T] == 2]
    nwod = max(1, len(duals))

    # de-interleave the hd dimension: [even dims, odd dims]
    perm_hd = np.concatenate([np.arange(0, HD, 2), np.arange(1, HD, 2)])

    def permute_heads(w, nh):
        w4 = w.reshape(E, DIM, nh, HD)
        return w4[:, :, :, perm_hd].reshape(E, DIM, nh * HD)

    wq_p = permute_heads(wq, HQ)
    wk_p = permute_heads(wk, HK)
    wv_p = permute_heads(wv, HK)
    wo4 = wo.reshape(E, HQ, HD, DIM)[:, :, perm_hd, :].reshape(E, HQ * HD, DIM)

    cosf = np.concatenate([freqs_cos, freqs_cos], axis=1)          # (SEQ, HD)
    sinf = np.concatenate([-freqs_sin, freqs_sin], axis=1)         # (SEQ, HD)
    qw = np.asarray(q_norm_w, np.float32)[:, perm_hd]              # (E, HD)
    kw = np.asarray(k_norm_w, np.float32)[:, perm_hd]

    in_maps = []
    for c in range(N_CORES):
        b, r = divmod(c, TP)
        P = perms[b]
        m = mids[b][P]                       # modality per permuted position
        qs = slice(r * DQ, (r + 1) * DQ)
        ks = slice(r * DKV, (r + 1) * DKV)
        w0c = np.concatenate([wq_p[0][:, qs], wk_p[0][:, ks], wv_p[0][:, ks]], axis=1)
        w1c = np.concatenate([wq_p[1][:, qs], wk_p[1][:, ks], wv_p[1][:, ks]], axis=1)

        # x^T chain tiles with per-expert zeroing on dual tiles
        xTb = x[b].T[:, P]                   # (dim, seq) permuted
        xt_list = []
        for T in range(NT):
            tile_x = xTb[:, 128 * T:128 * (T + 1)]   # (dim, 128)
            mt = m[128 * T:128 * (T + 1)]
            for e in chains[T]:
                if kinds[T] == 2:
                    tx = tile_x * (mt == e)[None, :]
                else:
                    tx = tile_x
                xt_list.append(tx.reshape(KT, 128, 128))
        xTc = np.stack(xt_list)              # (nch, KT, 128dim, 128tok)
        xTc = np.ascontiguousarray(xTc.transpose(0, 2, 1, 3))

        # per-token folded cos/sin (q and k norm weights)
        cq = cosf[P] * qw[m]
        sq = sinf[P] * qw[m]

        # in-group causal masks for the permuted order
        pos = (P % 512)
        dmv = np.zeros((NT, 128, 512), np.float32)
        for j in range(NT):
            gj = j // 4
            kpos = pos[128 * j:128 * (j + 1)]
            qpos = pos[512 * gj:512 * (gj + 1)]
            dmv[j] = (kpos[:, None] <= qpos[None, :])

        # wo dual masks (pre-broadcast rows)
        wodmv = np.zeros((nwod, 2, 128, 128), np.float32)
        for i, T in enumerate(duals):
            mt = m[128 * T:128 * (T + 1)]
            for e in (0, 1):
                wodmv[i, e] = np.tile((mt == e)[None, :].astype(np.float32),
                                      (128, 1))

        im = {
            "xT": xTc.astype(ml_dtypes.bfloat16),
            "w0": w0c.astype(ml_dtypes.bfloat16),
            "w1": w1c.astype(ml_dtypes.bfloat16),
            "wo0": wo4[0][r * DQ:(r + 1) * DQ, :].astype(ml_dtypes.bfloat16),
            "wo1": wo4[1][r * DQ:(r + 1) * DQ, :].astype(ml_dtypes.bfloat16),
            "cosq": np.ascontiguousarray(cq).astype(ml_dtypes.bfloat16),
            "sinq": np.ascontiguousarray(sq).astype(ml_dtypes.bfloat16),
            "dmin": dmv.astype(ml_dtypes.bfloat16),
            "wodm": wodmv.astype(ml_dtypes.bfloat16),
        }
        if has_qkw:
            im["cosk"] = np.ascontiguousarray(cosf[P] * kw[m]).astype(ml_dtypes.bfloat16)
            im["sink"] = np.ascontiguousarray(sinf[P] * kw[m]).astype(ml_dtypes.bfloat16)
        if has_anw:
            aw = np.asarray(attn_norm_w, np.float32)
            im["anw0"] = np.ascontiguousarray(aw[0:1])
            im["anwd"] = (aw[1] - aw[0]).reshape(1, DIM).copy()
            mf = np.zeros((128, len(CHUNKS)), np.float32)
            for ci, (t0, ntl) in enumerate(CHUNKS):
                nrow = ntl * 32
                t0tok = t0 * 128
                mf[0:nrow, ci] = m[t0tok + nrow * r: t0tok + nrow * (r + 1)]
            im["mfin"] = mf
        in_maps.append(im)
    return in_maps


def kernel(**inputs):
    q_norm_w = np.asarray(inputs["q_norm_w"], np.float32)
    k_norm_w = np.asarray(inputs["k_norm_w"], np.float32)
    attn_norm_w = np.asarray(inputs["attn_norm_w"], np.float32)
    has_qkw = not np.array_equal(q_norm_w, k_norm_w)
    has_anw = not np.all(attn_norm_w == 1.0)

    perms, kinds = _plan(inputs["modality_ids"])
    key = (has_qkw, has_anw, kinds)
    if key not in _BUILD_CACHE:
        _BUILD_CACHE[key] = build_nc(has_qkw, has_anw, kinds)
    nc = _BUILD_CACHE[key]

    in_maps = _prep_inputs(
        inputs["x"], inputs["freqs_cos"], inputs["freqs_sin"],
        inputs["wq"], inputs["wk"], inputs["wv"], inputs["wo"],
        q_norm_w, k_norm_w, attn_norm_w, inputs["modality_ids"],
        has_qkw, has_anw, perms, kinds)

    res = run_bass_kernel_spmd(nc, in_maps, core_ids=list(range(N_CORES)))

    out = np.empty((BS, SEQ, DIM), np.float32)
    for c in range(N_CORES):
        b, r = divmod(c, TP)
        P = perms[b]
        oc = res.results[c]["out"]          # (SEQ//4, DIM), permuted rows
        row = 0
        for ci, (t0, ntl) in enumerate(CHUNKS):
            nrow = ntl * 32
            t0tok = t0 * 128
            sel = P[t0tok + nrow * r: t0tok + nrow * (r + 1)]
            out[b, sel, :] = oc[row:row + nrow, :]
            row += nrow
    return out
